# revision 2
# baseline (speedup 1.0000x reference)
"""GatedGraphClassifier on 8 trn2 NeuronCores (Bass, stock instructions only).

Pipeline (one NEFF, one launch, SPMD on cores 0-7):
  - nodes degree-sorted by in-degree, round-robin sharded across cores; each
    core owns N/8 node slots laid out [128 partitions x 98 columns] in SBUF
  - per propagation step: each core computes its m = h @ W shard (H-major
    matmuls, PE transposes to node-major rows), AllGathers the full m table,
    then aggregates with one indirect-DMA gather-accumulate (cce add) per
    (column, pass): the j-th in-neighbor message of 128 nodes lands in the
    SBUF accumulator per instruction.  Degree sorting keeps per-column pass
    counts uniform so the instruction count is ~E/8/128 per core.
  - GRU cell on H-major [64 x 512] tiles (PE matmuls accumulate x/h
    contributions in PSUM, ACT applies sigmoid/tanh with per-partition bias)
  - graph mean-pool partials via one-hot matmuls on device; host sums the 8
    partials, divides by counts, and applies the tiny MLP head.

Falls back to a pure-numpy implementation if the trn2 toolchain is missing.
"""
import os
import sys
import numpy as np

XD, H, L, BLOCKS = 79, 64, 2, 4
NCORES = 8
P = 128
NSTEPS = BLOCKS * L


class _Cfg:
    def __init__(self, n, g):
        self.N, self.G = n, g
        per_core = -(-n // NCORES)
        self.C = -(-per_core // P)
        self.NL = P * self.C
        self.CM = self.C + 1
        self.SHARD = P * self.CM
        self.TABLE = NCORES * self.SHARD


def _host_prep(cfg, x, edge_index, batch):
    N, G, C, CM, SHARD = cfg.N, cfg.G, cfg.C, cfg.CM, cfg.SHARD
    src = np.asarray(edge_index[0], np.int64)
    dst = np.asarray(edge_index[1], np.int64)
    deg = np.bincount(dst, minlength=N)
    order = np.argsort(-deg, kind="stable")
    grank = np.empty(N, np.int64)
    grank[order] = np.arange(N)
    node_core = (grank % NCORES).astype(np.int64)
    node_lrank = (grank // NCORES).astype(np.int64)
    node_p = node_lrank % P
    node_c = node_lrank // P
    trow = node_core * SHARD + node_p * CM + node_c

    dmax = np.zeros(C, np.int64)
    for k in range(NCORES):
        m = node_core == k
        dpc = np.zeros((P, C), np.int64)
        dpc[node_p[m], node_c[m]] = deg[m]
        np.maximum(dmax, dpc.max(axis=0), out=dmax)
    pass_cols = np.repeat(np.arange(C), dmax)
    nidx = int(pass_cols.shape[0])
    col_start = np.zeros(C, np.int64)
    col_start[1:] = np.cumsum(dmax)[:-1]

    idx_all = np.empty((NCORES, P, nidx), np.int32)
    for k in range(NCORES):
        zr = (k * SHARD + np.arange(P) * CM + C).astype(np.int32)
        idx_all[k] = zr[:, None]
        m = node_core[dst] == k
        es, ed = src[m], dst[m]
        lp, lc = node_p[ed], node_c[ed]
        key = lc * P + lp
        eo = np.argsort(key, kind="stable")
        es, key = es[eo], key[eo]
        lp, lc = lp[eo], lc[eo]
        newgrp = np.empty(key.shape[0], bool)
        newgrp[0] = True
        newgrp[1:] = key[1:] != key[:-1]
        gstart = np.where(newgrp)[0]
        firsts = np.repeat(gstart, np.diff(np.append(gstart, key.shape[0])))
        j = np.arange(key.shape[0]) - firsts
        kcol = col_start[lc] + j
        idx_all[k, lp, kcol] = trow[es].astype(np.int32)

    x = np.asarray(x, np.float32)
    batch64 = np.asarray(batch, np.int64)
    xT = np.zeros((NCORES, XD, cfg.NL), np.float32)
    bslot = np.full((NCORES, P, C), float(G), np.float32)
    for k in range(NCORES):
        nodes = order[np.arange(k, N, NCORES)]
        r = np.arange(nodes.shape[0])
        xT[k, :, :nodes.shape[0]] = x[nodes].T
        bslot[k, r % P, r // P] = batch64[nodes].astype(np.float32)

    counts = np.bincount(batch64, minlength=G).astype(np.float32)
    return dict(idx_all=idx_all, pass_cols=pass_cols, nidx=nidx, xT=xT,
                bslot=bslot, counts=counts)


def _build_program(cfg, nidx, pass_cols):
    from concourse import bass, bacc, mybir, tile
    from concourse.masks import make_identity
    f32 = mybir.dt.float32
    i32 = mybir.dt.int32
    AF = mybir.ActivationFunctionType
    OP = mybir.AluOpType
    G, C, CM, NL, SHARD, TABLE = (cfg.G, cfg.C, cfg.CM, cfg.NL, cfg.SHARD,
                                  cfg.TABLE)
    NG4 = -(-G // P)

    nc = bacc.Bacc("TRN2", target_bir_lowering=False, debug=False,
                   num_devices=NCORES)

    t_xT = nc.dram_tensor("xT", [XD, NL], f32, kind="ExternalInput")
    t_idx = nc.dram_tensor("idx", [P, nidx], i32, kind="ExternalInput")
    t_bslot = nc.dram_tensor("bslot", [P, C], f32, kind="ExternalInput")
    t_iota = nc.dram_tensor("iota", [P, G], f32, kind="ExternalInput")
    t_wproj = nc.dram_tensor("wprojT", [XD, H], f32, kind="ExternalInput")
    t_bproj = nc.dram_tensor("bproj", [H, 1], f32, kind="ExternalInput")
    t_ggc = nc.dram_tensor("ggc", [H, BLOCKS * L * H], f32, kind="ExternalInput")
    t_wih = nc.dram_tensor("wihT", [H, BLOCKS * 3 * H], f32, kind="ExternalInput")
    t_whh = nc.dram_tensor("whhT", [H, BLOCKS * 3 * H], f32, kind="ExternalInput")
    t_brz = nc.dram_tensor("brz", [H, BLOCKS * 2], f32, kind="ExternalInput")
    t_bin = nc.dram_tensor("bin", [H, BLOCKS], f32, kind="ExternalInput")
    t_bhn = nc.dram_tensor("bhn", [H, BLOCKS], f32, kind="ExternalInput")
    t_out = nc.dram_tensor("pooled", [NG4 * P, H], f32, kind="ExternalOutput")

    cc_in = nc.dram_tensor("cc_in", [SHARD, H], f32, kind="Internal")
    cc_out = nc.dram_tensor("cc_out", [TABLE, H], f32, kind="Internal",
                            addr_space="Shared")

    hT = nc.alloc_sbuf_tensor("hT", [H, NL], f32)
    acc = nc.alloc_sbuf_tensor("acc", [P, C, H], f32)
    mbuf = nc.alloc_sbuf_tensor("mbuf", [P, CM, H], f32)
    idxs = nc.alloc_sbuf_tensor("idxs", [P, nidx], i32)
    s_wproj = nc.alloc_sbuf_tensor("s_wproj", [XD, H], f32)
    s_bproj = nc.alloc_sbuf_tensor("s_bproj", [H, 1], f32)
    s_ggc = nc.alloc_sbuf_tensor("s_ggc", [H, BLOCKS * L * H], f32)
    s_wih = nc.alloc_sbuf_tensor("s_wih", [H, BLOCKS * 3 * H], f32)
    s_whh = nc.alloc_sbuf_tensor("s_whh", [H, BLOCKS * 3 * H], f32)
    s_brz = nc.alloc_sbuf_tensor("s_brz", [H, BLOCKS * 2], f32)
    s_bin = nc.alloc_sbuf_tensor("s_bin", [H, BLOCKS], f32)
    s_bhn = nc.alloc_sbuf_tensor("s_bhn", [H, BLOCKS], f32)
    s_iota = nc.alloc_sbuf_tensor("s_iota", [P, G], f32)
    s_bslot = nc.alloc_sbuf_tensor("s_bslot", [P, C], f32)
    s_ident = nc.alloc_sbuf_tensor("s_ident", [P, P], f32)
    s_pool = nc.alloc_sbuf_tensor("s_pool", [P, NG4, H], f32)

    groups = [(i * 512, 512) for i in range(NL // 512)]
    if NL % 512:
        groups.append((NL - NL % 512, NL % 512))

    with tile.TileContext(nc) as tc:
        with (
            tc.tile_pool(name="sb", bufs=2) as sb,
            tc.tile_pool(name="ps_g", bufs=1, space="PSUM") as ps_g,
            tc.tile_pool(name="ps_m", bufs=1, space="PSUM") as ps_m,
            tc.tile_pool(name="ps_t", bufs=2, space="PSUM") as ps_t,
        ):
            nc.sync.dma_start(out=idxs[:, :], in_=t_idx[:, :])
            nc.sync.dma_start(out=s_wproj[:, :], in_=t_wproj[:, :])
            nc.sync.dma_start(out=s_bproj[:, :], in_=t_bproj[:, :])
            nc.sync.dma_start(out=s_ggc[:, :], in_=t_ggc[:, :])
            nc.sync.dma_start(out=s_wih[:, :], in_=t_wih[:, :])
            nc.sync.dma_start(out=s_whh[:, :], in_=t_whh[:, :])
            nc.sync.dma_start(out=s_brz[:, :], in_=t_brz[:, :])
            nc.sync.dma_start(out=s_bin[:, :], in_=t_bin[:, :])
            nc.sync.dma_start(out=s_bhn[:, :], in_=t_bhn[:, :])
            nc.sync.dma_start(out=s_iota[:, :], in_=t_iota[:, :])
            nc.sync.dma_start(out=s_bslot[:, :], in_=t_bslot[:, :])
            make_identity(nc, s_ident[:, :])
            nc.vector.memset(mbuf[:, :, :], 0.0)

            for g0, w in groups:
                xt = sb.tile([XD, 512], f32, tag="xt")
                nc.sync.dma_start(out=xt[:, :w], in_=t_xT[:, g0:g0 + w])
                h0 = ps_m.tile([H, 512], f32, tag="mm")
                nc.tensor.matmul(out=h0[:, :w], lhsT=s_wproj[:, :],
                                 rhs=xt[:, :w], start=True, stop=True)
                nc.scalar.activation(out=hT[:, g0:g0 + w], in_=h0[:, :w],
                                     func=AF.Identity, bias=s_bproj[:, 0:1])

            for step in range(NSTEPS):
                b, l = step // L, step % L
                wslice = s_ggc[:, (b * L + l) * H:(b * L + l + 1) * H]
                for g0, w in groups:
                    mp = ps_m.tile([H, 512], f32, tag="mm")
                    nc.tensor.matmul(out=mp[:, :w], lhsT=wslice,
                                     rhs=hT[:, g0:g0 + w], start=True, stop=True)
                    msb = sb.tile([H, 512], f32, tag="msb")
                    nc.vector.tensor_copy(out=msb[:, :w], in_=mp[:, :w])
                    for cb in range(w // P):
                        col = g0 // P + cb
                        tp = ps_t.tile([P, P], f32, tag="tp")
                        nc.tensor.transpose(out=tp[:, :H],
                                            in_=msb[:, cb * P:(cb + 1) * P],
                                            identity=s_ident[:H, :H])
                        nc.vector.tensor_copy(out=mbuf[:, col, :],
                                              in_=tp[:, :H])
                nc.sync.dma_start(out=cc_in[:, :], in_=mbuf[:, :, :])
                nc.gpsimd.collective_compute(
                    "AllGather", OP.bypass,
                    ins=[cc_in[:, :]], outs=[cc_out[:, :]],
                    replica_groups=[list(range(NCORES))],
                )
                nc.vector.memset(acc[:, :, :], 0.0)
                for k in range(nidx):
                    ck = int(pass_cols[k])
                    nc.gpsimd.indirect_dma_start(
                        out=acc[:, ck, :], out_offset=None,
                        in_=cc_out[:, :],
                        in_offset=bass.IndirectOffsetOnAxis(
                            ap=idxs[:, k:k + 1], axis=0),
                        compute_op=OP.add,
                    )
                w_ih = s_wih[:, b * 3 * H:(b + 1) * 3 * H]
                w_hh = s_whh[:, b * 3 * H:(b + 1) * 3 * H]
                for g0, w in groups:
                    hs = hT[:, g0:g0 + w]
                    at = sb.tile([H, 512], f32, tag="at")
                    for cb in range(w // P):
                        col = g0 // P + cb
                        tp = ps_t.tile([P, P], f32, tag="tp")
                        nc.tensor.transpose(out=tp[:H, :], in_=acc[:, col, :],
                                            identity=s_ident[:, :])
                        nc.vector.tensor_copy(out=at[:, cb * P:(cb + 1) * P],
                                              in_=tp[:H, :])
                    gr = ps_g.tile([H, 512], f32, tag="gr")
                    gz = ps_g.tile([H, 512], f32, tag="gz")
                    gni = ps_g.tile([H, 512], f32, tag="gni")
                    gnh = ps_g.tile([H, 512], f32, tag="gnh")
                    nc.tensor.matmul(out=gr[:, :w], lhsT=w_ih[:, 0:H],
                                     rhs=at[:, :w], start=True, stop=False)
                    nc.tensor.matmul(out=gr[:, :w], lhsT=w_hh[:, 0:H],
                                     rhs=hs, start=False, stop=True)
                    nc.tensor.matmul(out=gz[:, :w], lhsT=w_ih[:, H:2 * H],
                                     rhs=at[:, :w], start=True, stop=False)
                    nc.tensor.matmul(out=gz[:, :w], lhsT=w_hh[:, H:2 * H],
                                     rhs=hs, start=False, stop=True)
                    nc.tensor.matmul(out=gni[:, :w], lhsT=w_ih[:, 2 * H:3 * H],
                                     rhs=at[:, :w], start=True, stop=True)
                    nc.tensor.matmul(out=gnh[:, :w], lhsT=w_hh[:, 2 * H:3 * H],
                                     rhs=hs, start=True, stop=True)
                    r = sb.tile([H, 512], f32, tag="r")
                    z = sb.tile([H, 512], f32, tag="z")
                    hn = sb.tile([H, 512], f32, tag="hn")
                    ns = sb.tile([H, 512], f32, tag="ns")
                    nn = sb.tile([H, 512], f32, tag="nn")
                    hm = sb.tile([H, 512], f32, tag="hm")
                    nc.scalar.activation(out=r[:, :w], in_=gr[:, :w],
                                         func=AF.Sigmoid,
                                         bias=s_brz[:, 2 * b:2 * b + 1])
                    nc.scalar.activation(out=z[:, :w], in_=gz[:, :w],
                                         func=AF.Sigmoid,
                                         bias=s_brz[:, 2 * b + 1:2 * b + 2])
                    nc.scalar.activation(out=hn[:, :w], in_=gnh[:, :w],
                                         func=AF.Identity,
                                         bias=s_bhn[:, b:b + 1])
                    nc.vector.tensor_tensor(out=hn[:, :w], in0=r[:, :w],
                                            in1=hn[:, :w], op=OP.mult)
                    nc.vector.tensor_tensor(out=ns[:, :w], in0=gni[:, :w],
                                            in1=hn[:, :w], op=OP.add)
                    nc.scalar.activation(out=nn[:, :w], in_=ns[:, :w],
                                         func=AF.Tanh, bias=s_bin[:, b:b + 1])
                    nc.vector.tensor_tensor(out=hm[:, :w], in0=hs,
                                            in1=nn[:, :w], op=OP.subtract)
                    nc.vector.tensor_tensor(out=hm[:, :w], in0=z[:, :w],
                                            in1=hm[:, :w], op=OP.mult)
                    if l == L - 1:
                        nc.vector.tensor_tensor(out=hm[:, :w], in0=nn[:, :w],
                                                in1=hm[:, :w], op=OP.add)
                        nc.scalar.activation(out=hT[:, g0:g0 + w],
                                             in_=hm[:, :w], func=AF.Relu)
                    else:
                        nc.vector.tensor_tensor(out=hT[:, g0:g0 + w],
                                                in0=nn[:, :w], in1=hm[:, :w],
                                                op=OP.add)

            nc.vector.memset(s_pool[:, :, :], 0.0)
            for col in range(C):
                tp = ps_t.tile([P, P], f32, tag="tp")
                nc.tensor.transpose(out=tp[:, :H],
                                    in_=hT[:, col * P:(col + 1) * P],
                                    identity=s_ident[:H, :H])
                hnode = sb.tile([P, H], f32, tag="hnode")
                nc.vector.tensor_copy(out=hnode[:, :], in_=tp[:, :H])
                oh = sb.tile([P, G], f32, tag="oh")
                nc.vector.tensor_tensor(
                    out=oh[:, :],
                    in0=s_bslot[:, col:col + 1].to_broadcast([P, G]),
                    in1=s_iota[:, :], op=OP.is_equal)
                for gi in range(NG4):
                    gw = min(P, G - gi * P)
                    pp = ps_t.tile([P, P], f32, tag="tp")
                    nc.tensor.matmul(out=pp[:gw, :H],
                                     lhsT=oh[:, gi * P:gi * P + gw],
                                     rhs=hnode[:, :], start=True, stop=True)
                    nc.vector.tensor_tensor(out=s_pool[:gw, gi, :],
                                            in0=s_pool[:gw, gi, :],
                                            in1=pp[:gw, :H], op=OP.add)
            for gi in range(NG4):
                nc.sync.dma_start(out=t_out[gi * P:(gi + 1) * P, :],
                                  in_=s_pool[:, gi, :])
    nc.compile()
    return nc


def _make_in_maps(cfg, prep, W_proj, b_proj, ggc_W, gru_Wih, gru_Whh,
                  gru_bih, gru_bhh):
    G = cfg.G
    ggc = np.concatenate([ggc_W[b, l] for b in range(BLOCKS) for l in range(L)],
                         axis=1).astype(np.float32)
    wih = np.concatenate([np.asarray(gru_Wih[b]).T for b in range(BLOCKS)],
                         axis=1)
    whh = np.concatenate([np.asarray(gru_Whh[b]).T for b in range(BLOCKS)],
                         axis=1)
    bsum = np.asarray(gru_bih, np.float32) + np.asarray(gru_bhh, np.float32)
    brz = np.concatenate(
        [bsum[b, :2 * H].reshape(2, H).T for b in range(BLOCKS)], axis=1)
    bin_ = np.stack([np.asarray(gru_bih)[b, 2 * H:] for b in range(BLOCKS)],
                    axis=1)
    bhn = np.stack([np.asarray(gru_bhh)[b, 2 * H:] for b in range(BLOCKS)],
                   axis=1)
    iota = np.tile(np.arange(G, dtype=np.float32), (P, 1))
    common = dict(
        wprojT=np.ascontiguousarray(np.asarray(W_proj, np.float32).T),
        bproj=np.asarray(b_proj, np.float32).reshape(H, 1),
        ggc=np.ascontiguousarray(ggc, np.float32),
        wihT=np.ascontiguousarray(wih.astype(np.float32)),
        whhT=np.ascontiguousarray(whh.astype(np.float32)),
        brz=np.ascontiguousarray(brz.astype(np.float32)),
        bin=np.ascontiguousarray(bin_.astype(np.float32)),
        bhn=np.ascontiguousarray(bhn.astype(np.float32)),
        iota=np.ascontiguousarray(iota),
    )
    in_maps = []
    for k in range(NCORES):
        m = dict(common)
        m["xT"] = np.ascontiguousarray(prep["xT"][k])
        m["idx"] = np.ascontiguousarray(prep["idx_all"][k])
        m["bslot"] = np.ascontiguousarray(prep["bslot"][k])
        in_maps.append(m)
    return in_maps


def _finish(cfg, prep, pooled_parts, W1, b1, W2, b2, W3, b3):
    G = cfg.G
    pooled = np.zeros((G, H), np.float32)
    for part in pooled_parts:
        pooled += part[:G]
    pooled /= np.maximum(prep["counts"], 1.0)[:, None]
    out = pooled @ np.asarray(W1, np.float32).T + np.asarray(b1, np.float32)
    out = out @ np.asarray(W2, np.float32).T + np.asarray(b2, np.float32)
    out = out @ np.asarray(W3, np.float32).T + np.asarray(b3, np.float32)
    return (1.0 / (1.0 + np.exp(-out))).astype(np.float32)


def _kernel_numpy(x, edge_index, batch, W_proj, b_proj, ggc_W, gru_Wih,
                  gru_Whh, gru_bih, gru_bhh, W1, b1, W2, b2, W3, b3):
    """CPU fallback (exact, slower)."""
    N = x.shape[0]
    G = int(np.max(batch)) + 1 if batch.size else 1
    src, dst = edge_index[0].astype(np.int64), edge_index[1].astype(np.int64)
    order = np.argsort(dst, kind="stable")
    dst_s, src_s = dst[order], src[order]
    uniq_dst, seg_starts = np.unique(dst_s, return_index=True)
    h = np.asarray(x, np.float32) @ np.asarray(W_proj, np.float32).T + b_proj

    def sig(v):
        return 1.0 / (1.0 + np.exp(-v))

    for b in range(BLOCKS):
        WihT = np.ascontiguousarray(np.asarray(gru_Wih[b]).T)
        WhhT = np.ascontiguousarray(np.asarray(gru_Whh[b]).T)
        for l in range(L):
            m = h @ np.asarray(ggc_W[b, l], np.float32)
            sums = np.add.reduceat(m[src_s], seg_starts, axis=0)
            agg = np.zeros((N, H), np.float32)
            agg[uniq_dst] = sums
            gi = agg @ WihT + gru_bih[b]
            gh = h @ WhhT + gru_bhh[b]
            r = sig(gi[:, :H] + gh[:, :H])
            z = sig(gi[:, H:2 * H] + gh[:, H:2 * H])
            n = np.tanh(gi[:, 2 * H:] + r * gh[:, 2 * H:])
            h = (1.0 - z) * n + z * h
        h = np.maximum(h, 0.0)
    counts = np.bincount(batch, minlength=512).astype(np.float32)
    uniq_g, g_starts = np.unique(batch, return_index=True)
    pooled = np.zeros((512, H), np.float32)
    pooled[uniq_g] = np.add.reduceat(h, g_starts, axis=0)
    pooled /= np.maximum(counts, 1.0)[:, None]
    out = pooled @ np.asarray(W1, np.float32).T + b1
    out = out @ np.asarray(W2, np.float32).T + b2
    out = out @ np.asarray(W3, np.float32).T + b3
    return sig(out).astype(np.float32)


def kernel(x, edge_index, batch, W_proj, b_proj, ggc_W, gru_Wih, gru_Whh,
           gru_bih, gru_bhh, W1, b1, W2, b2, W3, b3):
    x = np.asarray(x)
    edge_index = np.asarray(edge_index)
    batch = np.asarray(batch)
    try:
        if "/opt/trn_rl_repo" not in sys.path:
            sys.path.insert(0, "/opt/trn_rl_repo")
        from concourse import bass_utils  # noqa: F401
    except Exception:
        return _kernel_numpy(x, edge_index, batch, W_proj, b_proj, ggc_W,
                             gru_Wih, gru_Whh, gru_bih, gru_bhh,
                             W1, b1, W2, b2, W3, b3)

    cfg = _Cfg(x.shape[0], 512)
    prep = _host_prep(cfg, x, edge_index, batch)
    nc = _build_program(cfg, prep["nidx"], prep["pass_cols"])
    in_maps = _make_in_maps(cfg, prep, W_proj, b_proj, ggc_W, gru_Wih,
                            gru_Whh, gru_bih, gru_bhh)
    from concourse import bass_utils
    res = bass_utils.run_bass_kernel_spmd(nc, in_maps,
                                          core_ids=list(range(NCORES)))
    parts = [res.results[k]["pooled"] for k in range(NCORES)]
    return _finish(cfg, prep, parts, W1, b1, W2, b2, W3, b3)


# revision 6
# speedup vs baseline: 2.3918x; 2.3918x over previous
"""GatedGraphClassifier on 8 trn2 NeuronCores (Bass, stock instructions only).

Pipeline (one NEFF, one launch, SPMD on cores 0-7):
  - nodes degree-sorted by in-degree, round-robin sharded across cores; each
    core owns N/8 node slots laid out [128 partitions x C columns] in SBUF
  - per propagation step: each core computes its m = h @ W shard (H-major
    matmuls, PE transposes to node-major rows), AllGathers the full m table,
    then aggregates with indirect-DMA gather-accumulates (cce add): one
    instruction adds the j-th in-neighbor message of 128 nodes (one column)
    into the SBUF accumulator.  Columns are grouped into bands of similar
    max-degree; each band runs a For_i replay loop whose body reloads the
    offset-staging tile (dynamic slice) and fires one static indirect DMA per
    column — replayed loop instructions are ~15x cheaper than unique ones in
    this environment.
  - GRU cell on H-major [64 x 512] tiles inside For_i loops (PE matmuls
    accumulate x/h gate contributions in PSUM, ACT applies sigmoid/tanh with
    per-partition bias)
  - graph mean-pool partials via one-hot matmuls on device; host sums the 8
    partials, divides by counts, and applies the tiny MLP head.

Falls back to a pure-numpy implementation if the trn2 toolchain is missing.
"""
import sys
import numpy as np

XD, H, L, BLOCKS = 79, 64, 2, 4
NCORES = 8
P = 128
NSTEPS = BLOCKS * L
UNROLL = 2          # gather-band replay unroll (double-buffered stages)


class _Cfg:
    def __init__(self, n, g):
        self.N, self.G = n, g
        per_core = -(-n // NCORES)
        c = -(-per_core // P)
        c += (-c) % 4                  # NL divisible by 512
        self.C = c
        self.NL = P * c
        self.CM = c + 1
        self.SHARD = P * self.CM
        self.TABLE = NCORES * self.SHARD


def _make_bands(dmax):
    """Split columns into bands of similar max-degree.

    Returns list of (c0, c1, D) with D = per-column pass count for the band,
    padded to a multiple of UNROLL.  Columns with dmax == 0 are dropped.
    """
    C = len(dmax)
    bands = []
    c0 = 0
    while c0 < C:
        if dmax[c0] == 0:
            c0 += 1
            continue
        D = int(dmax[c0])
        c1 = c0 + 1
        while c1 < C and dmax[c1] > 0:
            nd = max(D, int(dmax[c1]))
            width = c1 - c0 + 1
            waste = nd * width - int(dmax[c0:c1 + 1].sum())
            # keep reload DMAs wide enough (>=4 cols) and padding waste small
            if width > 4 and waste > max(8, int(0.06 * nd * width)):
                break
            D = nd
            c1 += 1
        c1 = min(max(c1, c0 + 4), C)
        D = int(dmax[c0:c1].max())
        Dp = -(-D // UNROLL) * UNROLL
        bands.append((c0, c1, Dp))
        c0 = c1
    return bands


def _host_prep(cfg, x, edge_index, batch):
    N, G, C, CM, SHARD = cfg.N, cfg.G, cfg.C, cfg.CM, cfg.SHARD
    src = np.asarray(edge_index[0], np.int64)
    dst = np.asarray(edge_index[1], np.int64)
    deg = np.bincount(dst, minlength=N)
    order = np.argsort(-deg, kind="stable")
    grank = np.empty(N, np.int64)
    grank[order] = np.arange(N)
    node_core = (grank % NCORES).astype(np.int64)
    node_lrank = (grank // NCORES).astype(np.int64)
    node_p = node_lrank % P
    node_c = node_lrank // P
    trow = node_core * SHARD + node_p * CM + node_c

    dmax = np.zeros(C, np.int64)
    for k in range(NCORES):
        m = node_core == k
        dpc = np.zeros((P, C), np.int64)
        dpc[node_p[m], node_c[m]] = deg[m]
        np.maximum(dmax, dpc.max(axis=0), out=dmax)

    bands = _make_bands(dmax)
    # idx layout: band b pass j column c -> base_b + j*W_b + (c - c0_b)
    base = {}
    tot = 0
    for (c0, c1, D) in bands:
        base[c0] = tot
        tot += D * (c1 - c0)
    col_band = np.full(C, -1, np.int64)
    col_base = np.zeros(C, np.int64)
    col_w = np.zeros(C, np.int64)
    col_off = np.zeros(C, np.int64)
    for (c0, c1, D) in bands:
        col_band[c0:c1] = c0
        col_base[c0:c1] = base[c0]
        col_w[c0:c1] = c1 - c0
        col_off[c0:c1] = np.arange(c1 - c0)

    idx_all = np.empty((NCORES, P, tot), np.int32)
    for k in range(NCORES):
        zr = (k * SHARD + np.arange(P) * CM + C).astype(np.int32)
        idx_all[k] = zr[:, None]
        m = node_core[dst] == k
        es, ed = src[m], dst[m]
        lp, lc = node_p[ed], node_c[ed]
        key = lc * P + lp
        eo = np.argsort(key, kind="stable")
        es, key = es[eo], key[eo]
        lp, lc = lp[eo], lc[eo]
        newgrp = np.empty(key.shape[0], bool)
        newgrp[0] = True
        newgrp[1:] = key[1:] != key[:-1]
        gstart = np.where(newgrp)[0]
        firsts = np.repeat(gstart, np.diff(np.append(gstart, key.shape[0])))
        j = np.arange(key.shape[0]) - firsts
        pos = col_base[lc] + j * col_w[lc] + col_off[lc]
        idx_all[k, lp, pos] = trow[es].astype(np.int32)

    x = np.asarray(x, np.float32)
    batch64 = np.asarray(batch, np.int64)
    xT = np.zeros((NCORES, XD, cfg.NL), np.float32)
    bslot = np.full((NCORES, P, C), float(G), np.float32)
    for k in range(NCORES):
        nodes = order[np.arange(k, N, NCORES)]
        r = np.arange(nodes.shape[0])
        xT[k, :, :nodes.shape[0]] = x[nodes].T
        bslot[k, r % P, r // P] = batch64[nodes].astype(np.float32)

    counts = np.bincount(batch64, minlength=G).astype(np.float32)
    return dict(idx_all=idx_all, bands=bands, nidx=tot, xT=xT,
                bslot=bslot, counts=counts)


def _build_program(cfg, nidx, bands):
    from concourse import bass, bacc, mybir, tile
    from concourse.bass import ds
    from concourse.masks import make_identity
    f32 = mybir.dt.float32
    i32 = mybir.dt.int32
    AF = mybir.ActivationFunctionType
    OP = mybir.AluOpType
    G, C, CM, NL, SHARD, TABLE = (cfg.G, cfg.C, cfg.CM, cfg.NL, cfg.SHARD,
                                  cfg.TABLE)
    NG4 = -(-G // P)
    NGRP = NL // 512

    nc = bacc.Bacc("TRN2", target_bir_lowering=False, debug=False,
                   num_devices=NCORES)

    t_xT = nc.dram_tensor("xT", [XD, NL], f32, kind="ExternalInput")
    t_idx = nc.dram_tensor("idx", [P, nidx], i32, kind="ExternalInput")
    t_bslot = nc.dram_tensor("bslot", [P, C], f32, kind="ExternalInput")
    t_iota = nc.dram_tensor("iota", [P, G], f32, kind="ExternalInput")
    t_wproj = nc.dram_tensor("wprojT", [XD, H], f32, kind="ExternalInput")
    t_bproj = nc.dram_tensor("bproj", [H, 1], f32, kind="ExternalInput")
    t_ggc = nc.dram_tensor("ggc", [H, BLOCKS * L * H], f32, kind="ExternalInput")
    t_wih = nc.dram_tensor("wihT", [H, BLOCKS * 3 * H], f32, kind="ExternalInput")
    t_whh = nc.dram_tensor("whhT", [H, BLOCKS * 3 * H], f32, kind="ExternalInput")
    t_brz = nc.dram_tensor("brz", [H, BLOCKS * 2], f32, kind="ExternalInput")
    t_bin = nc.dram_tensor("bin", [H, BLOCKS], f32, kind="ExternalInput")
    t_bhn = nc.dram_tensor("bhn", [H, BLOCKS], f32, kind="ExternalInput")
    t_out = nc.dram_tensor("pooled", [NG4 * P, H], f32, kind="ExternalOutput")

    cc_in = nc.dram_tensor("cc_in", [SHARD, H], f32, kind="Internal")
    cc_out = nc.dram_tensor("cc_out", [TABLE, H], f32, kind="Internal",
                            addr_space="Shared")

    hT = nc.alloc_sbuf_tensor("hT", [H, NL], f32)
    acc = nc.alloc_sbuf_tensor("acc", [P, C * H], f32)
    mbuf = nc.alloc_sbuf_tensor("mbuf", [P, CM * H], f32)
    stages = [nc.alloc_sbuf_tensor(f"stage{u}", [P, C], i32)
              for u in range(UNROLL)]
    s_wproj = nc.alloc_sbuf_tensor("s_wproj", [XD, H], f32)
    s_bproj = nc.alloc_sbuf_tensor("s_bproj", [H, 1], f32)
    s_ggc = nc.alloc_sbuf_tensor("s_ggc", [H, BLOCKS * L * H], f32)
    s_wih = nc.alloc_sbuf_tensor("s_wih", [H, BLOCKS * 3 * H], f32)
    s_whh = nc.alloc_sbuf_tensor("s_whh", [H, BLOCKS * 3 * H], f32)
    s_brz = nc.alloc_sbuf_tensor("s_brz", [H, BLOCKS * 2], f32)
    s_bin = nc.alloc_sbuf_tensor("s_bin", [H, BLOCKS], f32)
    s_bhn = nc.alloc_sbuf_tensor("s_bhn", [H, BLOCKS], f32)
    s_iota = nc.alloc_sbuf_tensor("s_iota", [P, G], f32)
    s_bslot = nc.alloc_sbuf_tensor("s_bslot", [P, C], f32)
    s_ident = nc.alloc_sbuf_tensor("s_ident", [P, P], f32)

    with tile.TileContext(nc) as tc:
        with (
            tc.tile_pool(name="sb", bufs=2) as sb,
            tc.tile_pool(name="ps_g", bufs=1, space="PSUM") as ps_g,
            tc.tile_pool(name="ps_m", bufs=1, space="PSUM") as ps_m,
            tc.tile_pool(name="ps_t", bufs=1, space="PSUM") as ps_t,
        ):
            nc.sync.dma_start(out=s_wproj[:, :], in_=t_wproj[:, :])
            nc.sync.dma_start(out=s_bproj[:, :], in_=t_bproj[:, :])
            nc.sync.dma_start(out=s_ggc[:, :], in_=t_ggc[:, :])
            nc.sync.dma_start(out=s_wih[:, :], in_=t_wih[:, :])
            nc.sync.dma_start(out=s_whh[:, :], in_=t_whh[:, :])
            nc.sync.dma_start(out=s_brz[:, :], in_=t_brz[:, :])
            nc.sync.dma_start(out=s_bin[:, :], in_=t_bin[:, :])
            nc.sync.dma_start(out=s_bhn[:, :], in_=t_bhn[:, :])
            nc.sync.dma_start(out=s_iota[:, :], in_=t_iota[:, :])
            nc.sync.dma_start(out=s_bslot[:, :], in_=t_bslot[:, :])
            make_identity(nc, s_ident[:, :])
            nc.vector.memset(mbuf[:, :], 0.0)

            # ---- input projection ----
            with tc.For_i(0, NGRP, 1) as g:
                xt = sb.tile([XD, 512], f32, tag="xt")
                nc.sync.dma_start(out=xt[:, :], in_=t_xT[:, ds(g * 512, 512)])
                h0 = ps_m.tile([H, 512], f32, tag="mm")
                nc.tensor.matmul(out=h0[:, :], lhsT=s_wproj[:, :],
                                 rhs=xt[:, :], start=True, stop=True)
                nc.scalar.activation(out=hT[:, ds(g * 512, 512)], in_=h0[:, :],
                                     func=AF.Identity, bias=s_bproj[:, 0:1])

            for step in range(NSTEPS):
                b, l = step // L, step % L
                wslice = s_ggc[:, (b * L + l) * H:(b * L + l + 1) * H]
                # ---- m phase ----
                with tc.For_i(0, NGRP, 1) as g:
                    mp = ps_m.tile([H, 512], f32, tag="mm")
                    nc.tensor.matmul(out=mp[:, :], lhsT=wslice,
                                     rhs=hT[:, ds(g * 512, 512)],
                                     start=True, stop=True)
                    msb = sb.tile([H, 512], f32, tag="msb")
                    nc.vector.tensor_copy(out=msb[:, :], in_=mp[:, :])
                    for cb in range(4):
                        tp = ps_t.tile([P, H], f32, tag=f"tp{cb % 2}")
                        nc.tensor.transpose(out=tp[:, :],
                                            in_=msb[:, cb * P:(cb + 1) * P],
                                            identity=s_ident[:H, :H])
                        nc.vector.tensor_copy(
                            out=mbuf[:, ds(g * 4 * H + cb * H, H)],
                            in_=tp[:, :])
                nc.sync.dma_start(out=cc_in[:, :], in_=mbuf[:, :])
                nc.gpsimd.collective_compute(
                    "AllGather", OP.bypass,
                    ins=[cc_in[:, :]], outs=[cc_out[:, :]],
                    replica_groups=[list(range(NCORES))],
                )
                # ---- aggregation (banded replay loops) ----
                nc.vector.memset(acc[:, :], 0.0)
                for (c0, c1, D) in bands:
                    W = c1 - c0
                    bb = 0
                    for (cc0, cc1, dd) in bands:
                        if cc0 == c0:
                            break
                        bb += dd * (cc1 - cc0)
                    with tc.For_i(0, D // UNROLL, 1) as rep:
                        for u in range(UNROLL):
                            nc.sync.dma_start(
                                out=stages[u][:, :W],
                                in_=t_idx[:, ds(rep * (UNROLL * W) + bb + u * W,
                                                W)])
                            for ci in range(W):
                                nc.gpsimd.indirect_dma_start(
                                    out=acc[:, (c0 + ci) * H:(c0 + ci + 1) * H],
                                    out_offset=None,
                                    in_=cc_out[:, :],
                                    in_offset=bass.IndirectOffsetOnAxis(
                                        ap=stages[u][:, ci:ci + 1], axis=0),
                                    compute_op=OP.add,
                                )
                # ---- GRU ----
                w_ih = s_wih[:, b * 3 * H:(b + 1) * 3 * H]
                w_hh = s_whh[:, b * 3 * H:(b + 1) * 3 * H]
                with tc.For_i(0, NGRP, 1) as g:
                    hsl = ds(g * 512, 512)
                    at = sb.tile([H, 512], f32, tag="at")
                    for cb in range(4):
                        a1 = sb.tile([P, H], f32, tag=f"a1{cb % 2}")
                        nc.vector.tensor_copy(
                            out=a1[:, :], in_=acc[:, ds(g * 4 * H + cb * H, H)])
                        tp = ps_t.tile([H, P], f32, tag=f"tp{cb % 2}")
                        nc.tensor.transpose(out=tp[:, :], in_=a1[:, :],
                                            identity=s_ident[:, :])
                        nc.vector.tensor_copy(out=at[:, cb * P:(cb + 1) * P],
                                              in_=tp[:, :])
                    gr = ps_g.tile([H, 512], f32, tag="gr")
                    gz = ps_g.tile([H, 512], f32, tag="gz")
                    gni = ps_g.tile([H, 512], f32, tag="gni")
                    gnh = ps_g.tile([H, 512], f32, tag="gnh")
                    nc.tensor.matmul(out=gr[:, :], lhsT=w_ih[:, 0:H],
                                     rhs=at[:, :], start=True, stop=False)
                    nc.tensor.matmul(out=gr[:, :], lhsT=w_hh[:, 0:H],
                                     rhs=hT[:, hsl], start=False, stop=True)
                    nc.tensor.matmul(out=gz[:, :], lhsT=w_ih[:, H:2 * H],
                                     rhs=at[:, :], start=True, stop=False)
                    nc.tensor.matmul(out=gz[:, :], lhsT=w_hh[:, H:2 * H],
                                     rhs=hT[:, hsl], start=False, stop=True)
                    nc.tensor.matmul(out=gni[:, :], lhsT=w_ih[:, 2 * H:3 * H],
                                     rhs=at[:, :], start=True, stop=True)
                    nc.tensor.matmul(out=gnh[:, :], lhsT=w_hh[:, 2 * H:3 * H],
                                     rhs=hT[:, hsl], start=True, stop=True)
                    r = sb.tile([H, 512], f32, tag="r")
                    z = sb.tile([H, 512], f32, tag="z")
                    hn = sb.tile([H, 512], f32, tag="hn")
                    ns = sb.tile([H, 512], f32, tag="ns")
                    nn = sb.tile([H, 512], f32, tag="nn")
                    hm = sb.tile([H, 512], f32, tag="hm")
                    nc.scalar.activation(out=r[:, :], in_=gr[:, :],
                                         func=AF.Sigmoid,
                                         bias=s_brz[:, 2 * b:2 * b + 1])
                    nc.scalar.activation(out=z[:, :], in_=gz[:, :],
                                         func=AF.Sigmoid,
                                         bias=s_brz[:, 2 * b + 1:2 * b + 2])
                    nc.scalar.activation(out=hn[:, :], in_=gnh[:, :],
                                         func=AF.Identity,
                                         bias=s_bhn[:, b:b + 1])
                    nc.vector.tensor_tensor(out=hn[:, :], in0=r[:, :],
                                            in1=hn[:, :], op=OP.mult)
                    nc.vector.tensor_tensor(out=ns[:, :], in0=gni[:, :],
                                            in1=hn[:, :], op=OP.add)
                    nc.scalar.activation(out=nn[:, :], in_=ns[:, :],
                                         func=AF.Tanh, bias=s_bin[:, b:b + 1])
                    nc.vector.tensor_tensor(out=hm[:, :], in0=hT[:, hsl],
                                            in1=nn[:, :], op=OP.subtract)
                    nc.vector.tensor_tensor(out=hm[:, :], in0=z[:, :],
                                            in1=hm[:, :], op=OP.mult)
                    if l == L - 1:
                        nc.vector.tensor_tensor(out=hm[:, :], in0=nn[:, :],
                                                in1=hm[:, :], op=OP.add)
                        nc.scalar.activation(out=hT[:, hsl], in_=hm[:, :],
                                             func=AF.Relu)
                    else:
                        nc.vector.tensor_tensor(out=hT[:, hsl], in0=nn[:, :],
                                                in1=hm[:, :], op=OP.add)

            # ---- pooling ----
            # pool tile, not alloc_sbuf_tensor: loop-carried in-place engine
            # accumulation silently drops updates on raw SBUF tensors
            s_pool = sb.tile([P, NG4 * H], f32, tag="spool")
            nc.vector.memset(s_pool[:, :], 0.0)
            with tc.For_i(0, C, 1) as col:
                hsb = sb.tile([H, P], f32, tag="hsb")
                nc.vector.tensor_copy(out=hsb[:, :], in_=hT[:, ds(col * P, P)])
                tp = ps_t.tile([P, H], f32, tag="tp0")
                nc.tensor.transpose(out=tp[:, :], in_=hsb[:, :],
                                    identity=s_ident[:H, :H])
                hnode = sb.tile([P, H], f32, tag="hnode")
                nc.vector.tensor_copy(out=hnode[:, :], in_=tp[:, :])
                bst = sb.tile([P, 1], f32, tag="bst")
                nc.vector.tensor_copy(out=bst[:, :], in_=s_bslot[:, ds(col, 1)])
                oh = sb.tile([P, G], f32, tag="oh")
                nc.vector.tensor_tensor(out=oh[:, :],
                                        in0=bst[:, 0:1].to_broadcast([P, G]),
                                        in1=s_iota[:, :], op=OP.is_equal)
                for gi in range(NG4):
                    gw = min(P, G - gi * P)
                    pp = ps_t.tile([P, H], f32, tag="tp1")
                    nc.tensor.matmul(out=pp[:gw, :],
                                     lhsT=oh[:, gi * P:gi * P + gw],
                                     rhs=hnode[:, :], start=True, stop=True)
                    nc.vector.tensor_tensor(
                        out=s_pool[:gw, gi * H:(gi + 1) * H],
                        in0=s_pool[:gw, gi * H:(gi + 1) * H],
                        in1=pp[:gw, :], op=OP.add)
            ob = sb.tile([P, NG4 * H], f32, tag="spool_out")
            nc.vector.tensor_copy(out=ob[:, :], in_=s_pool[:, :])
            for gi in range(NG4):
                nc.sync.dma_start(out=t_out[gi * P:(gi + 1) * P, :],
                                  in_=ob[:, gi * H:(gi + 1) * H])
    nc.compile()
    return nc


def _make_in_maps(cfg, prep, W_proj, b_proj, ggc_W, gru_Wih, gru_Whh,
                  gru_bih, gru_bhh):
    G = cfg.G
    ggc = np.concatenate([ggc_W[b, l] for b in range(BLOCKS) for l in range(L)],
                         axis=1).astype(np.float32)
    wih = np.concatenate([np.asarray(gru_Wih[b]).T for b in range(BLOCKS)],
                         axis=1)
    whh = np.concatenate([np.asarray(gru_Whh[b]).T for b in range(BLOCKS)],
                         axis=1)
    bsum = np.asarray(gru_bih, np.float32) + np.asarray(gru_bhh, np.float32)
    brz = np.concatenate(
        [bsum[b, :2 * H].reshape(2, H).T for b in range(BLOCKS)], axis=1)
    bin_ = np.stack([np.asarray(gru_bih)[b, 2 * H:] for b in range(BLOCKS)],
                    axis=1)
    bhn = np.stack([np.asarray(gru_bhh)[b, 2 * H:] for b in range(BLOCKS)],
                   axis=1)
    iota = np.tile(np.arange(G, dtype=np.float32), (P, 1))
    common = dict(
        wprojT=np.ascontiguousarray(np.asarray(W_proj, np.float32).T),
        bproj=np.asarray(b_proj, np.float32).reshape(H, 1),
        ggc=np.ascontiguousarray(ggc, np.float32),
        wihT=np.ascontiguousarray(wih.astype(np.float32)),
        whhT=np.ascontiguousarray(whh.astype(np.float32)),
        brz=np.ascontiguousarray(brz.astype(np.float32)),
        bin=np.ascontiguousarray(bin_.astype(np.float32)),
        bhn=np.ascontiguousarray(bhn.astype(np.float32)),
        iota=np.ascontiguousarray(iota),
    )
    in_maps = []
    for k in range(NCORES):
        m = dict(common)
        m["xT"] = np.ascontiguousarray(prep["xT"][k])
        m["idx"] = np.ascontiguousarray(prep["idx_all"][k])
        m["bslot"] = np.ascontiguousarray(prep["bslot"][k])
        in_maps.append(m)
    return in_maps


def _finish(cfg, prep, pooled_parts, W1, b1, W2, b2, W3, b3):
    G = cfg.G
    pooled = np.zeros((G, H), np.float32)
    for part in pooled_parts:
        pooled += part[:G]
    pooled /= np.maximum(prep["counts"], 1.0)[:, None]
    out = pooled @ np.asarray(W1, np.float32).T + np.asarray(b1, np.float32)
    out = out @ np.asarray(W2, np.float32).T + np.asarray(b2, np.float32)
    out = out @ np.asarray(W3, np.float32).T + np.asarray(b3, np.float32)
    return (1.0 / (1.0 + np.exp(-out))).astype(np.float32)


def _kernel_numpy(x, edge_index, batch, W_proj, b_proj, ggc_W, gru_Wih,
                  gru_Whh, gru_bih, gru_bhh, W1, b1, W2, b2, W3, b3):
    """CPU fallback (exact, slower)."""
    N = x.shape[0]
    src, dst = edge_index[0].astype(np.int64), edge_index[1].astype(np.int64)
    order = np.argsort(dst, kind="stable")
    dst_s, src_s = dst[order], src[order]
    uniq_dst, seg_starts = np.unique(dst_s, return_index=True)
    h = np.asarray(x, np.float32) @ np.asarray(W_proj, np.float32).T + b_proj

    def sig(v):
        return 1.0 / (1.0 + np.exp(-v))

    for b in range(BLOCKS):
        WihT = np.ascontiguousarray(np.asarray(gru_Wih[b]).T)
        WhhT = np.ascontiguousarray(np.asarray(gru_Whh[b]).T)
        for l in range(L):
            m = h @ np.asarray(ggc_W[b, l], np.float32)
            sums = np.add.reduceat(m[src_s], seg_starts, axis=0)
            agg = np.zeros((N, H), np.float32)
            agg[uniq_dst] = sums
            gi = agg @ WihT + gru_bih[b]
            gh = h @ WhhT + gru_bhh[b]
            r = sig(gi[:, :H] + gh[:, :H])
            z = sig(gi[:, H:2 * H] + gh[:, H:2 * H])
            n = np.tanh(gi[:, 2 * H:] + r * gh[:, 2 * H:])
            h = (1.0 - z) * n + z * h
        h = np.maximum(h, 0.0)
    counts = np.bincount(batch, minlength=512).astype(np.float32)
    uniq_g, g_starts = np.unique(batch, return_index=True)
    pooled = np.zeros((512, H), np.float32)
    pooled[uniq_g] = np.add.reduceat(h, g_starts, axis=0)
    pooled /= np.maximum(counts, 1.0)[:, None]
    out = pooled @ np.asarray(W1, np.float32).T + b1
    out = out @ np.asarray(W2, np.float32).T + b2
    out = out @ np.asarray(W3, np.float32).T + b3
    return sig(out).astype(np.float32)


def kernel(x, edge_index, batch, W_proj, b_proj, ggc_W, gru_Wih, gru_Whh,
           gru_bih, gru_bhh, W1, b1, W2, b2, W3, b3):
    x = np.asarray(x)
    edge_index = np.asarray(edge_index)
    batch = np.asarray(batch)
    try:
        if "/opt/trn_rl_repo" not in sys.path:
            sys.path.insert(0, "/opt/trn_rl_repo")
        from concourse import bass_utils  # noqa: F401
    except Exception:
        return _kernel_numpy(x, edge_index, batch, W_proj, b_proj, ggc_W,
                             gru_Wih, gru_Whh, gru_bih, gru_bhh,
                             W1, b1, W2, b2, W3, b3)

    cfg = _Cfg(x.shape[0], 512)
    prep = _host_prep(cfg, x, edge_index, batch)
    nc = _build_program(cfg, prep["nidx"], prep["bands"])
    in_maps = _make_in_maps(cfg, prep, W_proj, b_proj, ggc_W, gru_Wih,
                            gru_Whh, gru_bih, gru_bhh)
    from concourse import bass_utils
    res = bass_utils.run_bass_kernel_spmd(nc, in_maps,
                                          core_ids=list(range(NCORES)))
    parts = [res.results[k]["pooled"] for k in range(NCORES)]
    return _finish(cfg, prep, parts, W1, b1, W2, b2, W3, b3)
